# revision 53
# baseline (speedup 1.0000x reference)
"""Bass/Trainium2 kernel for nn_DTree (soft decision tree MoE routing).

Contract: kernel(**inputs) takes the FULL unsharded inputs (np/jax arrays,
keyed as in setup_inputs()) and returns the FULL [8192, 256] float32 output.

Strategy: pure data parallelism over 8 NeuronCores - the batch (8192) is
sharded 1024 rows/core, all parameters are replicated, and the single
cross-batch quantity (mean entropy -> output scale) is combined with one
32-byte AllReduce that overlaps with the main leaf matmul.

v2: the dominant leaf matmul ([1024,512]@[512,32768] per core) runs in
fp8-e4m3 DoubleRow mode (K=256 per instruction, 2 rows/cycle) - 4x fewer
PE cycles than bf16. The fp8 quantization noise is cancelled to first
order by an exact-mean correction: corr = h@Wbar - h8@W8bar, computed as
two tiny bf16 matmuls (h@CA + dh@CB) folded into the per-row-tile
accumulator init. The per-leaf weighted combine (acc += y_leaf * s_w) is
spread across three engine paths: DVE stt / Pool stt (SBUF accumulators)
and ACT-mult + PE-identity-add (PSUM accumulators), balanced so no engine
exceeds the PE's matmul time.
"""

import sys

for _p in ("/opt/trn_rl_repo",):
    if _p not in sys.path:
        sys.path.insert(0, _p)

import numpy as np
import ml_dtypes

import concourse.bass as bass
import concourse.mybir as mybir
import concourse.tile as tile
from concourse import bacc
from concourse.bass_utils import run_bass_kernel_spmd
from concourse.masks import make_identity

F32 = mybir.dt.float32
BF16 = mybir.dt.bfloat16
FP8 = mybir.dt.float8e4
AF = mybir.ActivationFunctionType
ALU = mybir.AluOpType
DR = mybir.MatmulPerfMode.DoubleRow

N_CORES = 8
BATCH = 8192
B_CORE = BATCH // N_CORES  # 1024
D_IN = 512
D1 = 513
NUM_NODES = 127
NUM_LEAVES = 128
D_OUT = 256
DEPTH = 6
EPS = 0.01
MAX_ENT = (NUM_LEAVES / DEPTH) * float(np.log(DEPTH))
LN128 = float(np.log(NUM_LEAVES))  # leaves_logp shift from centered tA/tB
W_SCALE = 64.0  # host-side scale on W_leaf before fp8 quantization

N_BT = B_CORE // 128          # 8 batch tiles per core
N_KT = D_IN // 128            # 4 contraction tiles (d = 0..511)
N_NC = (NUM_LEAVES * D_OUT) // 512  # 64 leaf column chunks of 512 (2 leaves)
NB2 = B_CORE // 512           # 2 batch halves of 512
N_EARLY = 0                   # leaf chunks issued before the routing chain

# combine path assignment, uniform per chunk (16 ops over 4 engines):
# bt 6,7 -> both ops ACT mult + PE ident-add (PSUM accumulators accP6/7);
# bt 5   -> leaf A on ACT (accP5), leaf B on DVE (acc_d);
# bt 0-4 -> leaf A on DVE (acc_d), leaf B on Pool (acc_p).
# => per chunk: DVE 6 ops, Pool 5, ACT 5, PE 5 ident-adds.
ACT_BTS = (6, 7)
ADD_LAG = 3     # PE ident-adds run this many chunks behind their matmuls
BT_ORDER = (0, 1, 2, 5, 3, 6, 4, 7)  # production order: slot reuse aligned with consumer completion
PA_BTS = (4, 7)  # last-produced tiles get the phase-A psum pool


_COMPILED = None


def _build_program(with_collective=True):
    nc = bacc.Bacc("TRN2", target_bir_lowering=False, debug=False,
                   num_devices=N_CORES if with_collective else 1)

    # ---- per-core DRAM I/O ----
    xT_c = nc.dram_tensor("xT_c", [D_IN, B_CORE], BF16, kind="ExternalInput")
    wpre = nc.dram_tensor("wpre", [D_IN, D1], BF16, kind="ExternalInput")
    bpre = nc.dram_tensor("bpre", [D1, 1], F32, kind="ExternalInput")
    nwt = nc.dram_tensor("nwt", [D1, NUM_NODES], BF16, kind="ExternalInput")
    mroute = nc.dram_tensor("mroute", [256, 128], BF16, kind="ExternalInput")
    wleaf8 = nc.dram_tensor("wleaf8", [D_IN, NUM_LEAVES * D_OUT], FP8,
                            kind="ExternalInput")
    wbias = nc.dram_tensor("wbias", [NUM_LEAVES, D_OUT], BF16,
                           kind="ExternalInput")
    wcol512 = nc.dram_tensor("wcol512", [NUM_LEAVES, D_OUT], BF16,
                             kind="ExternalInput")
    corr_a = nc.dram_tensor("corr_a", [D_IN, D_OUT], BF16,
                            kind="ExternalInput")
    corr_b = nc.dram_tensor("corr_b", [D_IN, D_OUT], BF16,
                            kind="ExternalInput")
    out_c = nc.dram_tensor("out_c", [B_CORE, D_OUT], F32,
                           kind="ExternalOutput")

    with tile.TileContext(nc) as tc:
        from contextlib import ExitStack
        with ExitStack() as ctx:
            const = ctx.enter_context(tc.tile_pool(name="const", bufs=1))
            work = ctx.enter_context(tc.tile_pool(name="work", bufs=1))
            dram = ctx.enter_context(tc.tile_pool(name="dram", bufs=1,
                                                  space="DRAM"))

            # ---- constants into SBUF ----
            # per-kt tiles: A1's first matmul starts as soon as the first
            # 128-row slices of W_pre and x land, not after the full DMA
            wp_sb, xt_sb = [], []
            for kt in range(N_KT):
                w_t = const.tile([128, D1], BF16, tag=f"wp{kt}",
                                 name=f"wp{kt}")
                nc.sync.dma_start(w_t[:, :], wpre[kt * 128:(kt + 1) * 128, :])
                x_t = const.tile([128, B_CORE], BF16, tag=f"xt{kt}",
                                 name=f"xt{kt}")
                nc.sync.dma_start(x_t[:, :], xT_c[kt * 128:(kt + 1) * 128, :])
                wp_sb.append(w_t)
                xt_sb.append(x_t)

            bpre_sb = const.tile([128, 5], F32, tag="bpre")
            for mc in range(5):
                pc = 128 if mc < 4 else 1
                nc.sync.dma_start(bpre_sb[0:pc, mc:mc + 1],
                                  bpre[mc * 128:mc * 128 + pc, :])

            nwt_sb = const.tile([128, N_KT * NUM_NODES], BF16, tag="nwt")
            for kt in range(N_KT):
                nc.sync.dma_start(
                    nwt_sb[:, kt * NUM_NODES:(kt + 1) * NUM_NODES],
                    nwt[kt * 128:(kt + 1) * 128, :])
            nwt_last = const.tile([1, NUM_NODES], BF16, tag="nwtl")
            nc.sync.dma_start(nwt_last[0:1, :], nwt[512:513, :])

            m_sb = const.tile([128, 256], BF16, tag="m")
            nc.sync.dma_start(m_sb[:, 0:128], mroute[0:128, :])
            nc.sync.dma_start(m_sb[:, 128:256], mroute[128:256, :])

            wb_sb = const.tile([128, D_OUT], BF16, tag="wb")
            nc.sync.dma_start(wb_sb[:, :], wbias[:, :])
            wc5_sb = const.tile([128, D_OUT], BF16, tag="wc5")
            nc.sync.dma_start(wc5_sb[:, :], wcol512[:, :])
            ca_sb = const.tile([128, N_KT * D_OUT], BF16, tag="ca")
            cb_sb = const.tile([128, N_KT * D_OUT], BF16, tag="cb")
            for kt in range(N_KT):
                nc.sync.dma_start(ca_sb[:, kt * D_OUT:(kt + 1) * D_OUT],
                                  corr_a[kt * 128:(kt + 1) * 128, :])
                nc.sync.dma_start(cb_sb[:, kt * D_OUT:(kt + 1) * D_OUT],
                                  corr_b[kt * 128:(kt + 1) * 128, :])

            ident = const.tile([128, 128], F32, tag="ident")
            make_identity(nc, ident[:, :])
            identb = const.tile([128, 128], BF16, tag="identb")
            nc.scalar.activation(identb[:, :], ident[:, :], AF.Copy)
            ones_col = const.tile([128, 1], F32, tag="onesc")
            nc.vector.memset(ones_col[:, :], 1.0)
            ones_row = const.tile([1, 128], F32, tag="onesr")
            nc.vector.memset(ones_row[:, :], 1.0)
            zero_col = const.tile([128, 1], F32, tag="zeroc")
            nc.vector.memset(zero_col[:, :], 0.0)
            half_col = const.tile([128, 1], F32, tag="halfc")
            nc.vector.memset(half_col[:, :], 0.5)
            ones_col_b = const.tile([128, 1], BF16, tag="onescb")
            nc.vector.memset(ones_col_b[:, :], 1.0)
            ones_row_b = const.tile([1, 128], BF16, tag="onesrb")
            nc.vector.memset(ones_row_b[:, :], 1.0)
            nl128_col = const.tile([128, 1], F32, tag="nl128")
            nc.vector.memset(nl128_col[:, :], -LN128)

            # ---- persistent intermediates ----
            hTb = work.tile([128, N_KT * B_CORE], BF16, tag="hTb")
            hT8 = work.tile([128, N_KT, B_CORE], FP8, tag="hT8")
            h512b = work.tile([1, B_CORE], BF16, tag="h512b")
            dh = work.tile([128, N_KT * B_CORE], BF16, tag="dh")
            sq = work.tile([128, N_KT * B_CORE], BF16, tag="sq")
            sq_last = work.tile([1, B_CORE], BF16, tag="sql")
            rhh_ln = work.tile([1, B_CORE], F32, tag="rhhln")
            rhh_row_b = work.tile([1, B_CORE], BF16, tag="rhhrb")
            rhh_bc = work.tile([128, B_CORE], F32, tag="rhhbc")
            h512bc = work.tile([128, B_CORE], F32, tag="h512bc")
            cT = work.tile([128, B_CORE], F32, tag="cT")
            tA = work.tile([128, B_CORE], BF16, tag="tA")
            tB = work.tile([128, B_CORE], BF16, tag="tB")
            swT = work.tile([128, B_CORE], F32, tag="swT")
            swTb = work.tile([128, B_CORE], BF16, tag="swTb")
            sw512Tb = work.tile([128, B_CORE], BF16, tag="sw512Tb")
            entt = work.tile([128, B_CORE], BF16, tag="entt")
            sw_all = work.tile([128, B_CORE], F32, tag="swall")
            srow = work.tile([1, 8], F32, tag="srow")
            stot = work.tile([1, 8], F32, tag="stot")
            scal = work.tile([1, 1], F32, tag="scal")
            scol = work.tile([128, 1], F32, tag="scol")
            acc_d = work.tile([128, N_BT * D_OUT], F32, tag="accd")
            acc_p = work.tile([128, N_BT * D_OUT], F32, tag="accp")

            # ---- PSUM pools: 3 (leaf) + 2 (phase A) + 1 (init) + 2 (accP) ----
            ypool = ctx.enter_context(
                tc.tile_pool(name="ypool", bufs=3, space="PSUM"))
            initpool = ctx.enter_context(
                tc.tile_pool(name="initpool", bufs=1, space="PSUM"))
            papool = ctx.enter_context(
                tc.tile_pool(name="papool", bufs=2, space="PSUM"))
            accpool = ctx.enter_context(
                tc.tile_pool(name="accpool", bufs=1, space="PSUM"))
            wpool = ctx.enter_context(tc.tile_pool(name="wpool", bufs=10))
            opool = ctx.enter_context(tc.tile_pool(name="opool", bufs=2))
            tpool = ctx.enter_context(tc.tile_pool(name="tpool", bufs=24))

            # row 127 of tA/tB must be 0 (M rows 127/255 are zero)
            nc.vector.memset(tA[:, :], 0.0)
            nc.vector.memset(tB[:, :], 0.0)
            nc.vector.memset(srow[0:1, :], 0.0)
            nc.gpsimd.memset(acc_p[:, :], 0.0)

            # PE warm-up while input DMAs land (clock p-state ramp).
            pwarm = papool.tile([128, 128], F32, tag="pa")
            for i in range(12):
                nc.tensor.matmul(pwarm[:, :], ident[:, :], ident[:, :],
                                 start=(i == 0), stop=(i == 11))
            nc.scalar.activation(srow[0:1, 2:3], pwarm[0:1, 0:1], AF.Copy)
            # preload the natural-log activation table off the critical path
            nc.scalar.activation(srow[0:1, 3:4], ones_row[0:1, 0:1], AF.Ln,
                                 bias=zero_col[0:1, 0:1])

            # persistent PSUM accumulators for the ACT+PE combine path
            accP = {bt: accpool.tile([128, D_OUT], F32, tag=f"accP{bt}",
                                     name=f"accP{bt}")
                    for bt in ACT_BTS}

            # ======== phase A1: pre-network =============================
            def stepA1(ns):
                nsl = slice(ns * 512, (ns + 1) * 512)
                for mc in range(5):
                    pc = 128 if mc < 4 else 1
                    ph = papool.tile([128, 512], F32, tag="pa")
                    for kt in range(N_KT):
                        nc.tensor.matmul(
                            ph[0:pc, :],
                            wp_sb[kt][:, mc * 128:mc * 128 + pc],
                            xt_sb[kt][:, ns * 512:(ns + 1) * 512],
                            start=(kt == 0), stop=(kt == N_KT - 1))
                    if mc < 4:
                        csl = slice(mc * B_CORE + ns * 512,
                                    mc * B_CORE + (ns + 1) * 512)
                        nc.scalar.activation(hTb[:, csl], ph[0:pc, :], AF.Relu,
                                             bias=bpre_sb[0:pc, mc:mc + 1])
                        # fp8 relu directly from psum on DVE (parallel w/ ACT)
                        nc.vector.tensor_scalar(
                            hT8[:, mc, ns * 512:(ns + 1) * 512], ph[0:pc, :],
                            bpre_sb[0:pc, mc:mc + 1], 0.0,
                            op0=ALU.add, op1=ALU.max)
                    else:
                        nc.scalar.activation(h512b[0:1, nsl], ph[0:pc, :],
                                             AF.Relu,
                                             bias=bpre_sb[0:pc, mc:mc + 1])

            # ======== leaf matmul chunk (fp8 DoubleRow) =================
            def leaf_matmuls(ncx):
                w8 = wpool.tile([128, N_KT, 512], FP8, tag="w8")
                for j in range(N_KT):
                    nc.sync.dma_start(
                        w8[:, j, :],
                        wleaf8[j * 128:(j + 1) * 128,
                               ncx * 512:(ncx + 1) * 512])
                pys = [None] * N_BT
                for bt in BT_ORDER:
                    if bt in PA_BTS:
                        py = papool.tile([128, 512], F32, tag="pa",
                                         name="pyp")
                    else:
                        py = ypool.tile([128, 512], F32, tag="y", name="py")
                    for k2 in range(2):
                        nc.tensor.matmul(
                            py[:, :],
                            hT8[:, 2 * k2:2 * k2 + 2,
                                bt * 128:(bt + 1) * 128],
                            w8[:, 2 * k2:2 * k2 + 2, :],
                            start=(k2 == 0), stop=(k2 == 1),
                            perf_mode=DR)
                    pys[bt] = py
                return pys

            # ======== combine for one chunk =============================
            # ACT-path ident-adds are deferred ADD_LAG chunks so the PE
            # queue never waits on same-chunk ACT mults.
            pending_adds = {}  # ncx -> list of (dst, tmp, start, stop)

            def combine(ncx, bt, py, accP5):
                l0 = 2 * ncx
                swA = sw_all[:, bt * 128 + l0:bt * 128 + l0 + 1]
                swB = sw_all[:, bt * 128 + l0 + 1:bt * 128 + l0 + 2]
                asl_d = acc_d[:, bt * D_OUT:(bt + 1) * D_OUT]
                asl_p = acc_p[:, bt * D_OUT:(bt + 1) * D_OUT]
                if bt == 1:
                    # ACT weights both leaves into bf16 tmps; Pool
                    # (SBUF-only tensor_tensor) accumulates into acc_p
                    tmpA = tpool.tile([128, D_OUT], BF16, tag="tmp")
                    tmpB = tpool.tile([128, D_OUT], BF16, tag="tmp")
                    nc.scalar.activation(tmpA[:, :], py[:, 0:D_OUT],
                                         AF.Copy, scale=swA)
                    nc.scalar.activation(tmpB[:, :], py[:, D_OUT:2 * D_OUT],
                                         AF.Copy, scale=swB)
                    nc.gpsimd.tensor_tensor(asl_p, tmpA[:, :], asl_p,
                                            op=ALU.add)
                    nc.gpsimd.tensor_tensor(asl_p, tmpB[:, :], asl_p,
                                            op=ALU.add)
                elif bt == 0 or bt < 5:  # DVE fused stt from psum
                    nc.vector.scalar_tensor_tensor(
                        asl_d, py[:, 0:D_OUT], swA, asl_d,
                        op0=ALU.mult, op1=ALU.add)
                    nc.vector.scalar_tensor_tensor(
                        asl_d, py[:, D_OUT:2 * D_OUT], swB, asl_d,
                        op0=ALU.mult, op1=ALU.add)
                else:  # ACT mult + deferred PE identity-add into PSUM acc
                    dst = accP5 if bt == 5 else accP[bt]
                    tmpA = tpool.tile([128, D_OUT], BF16, tag="tmp")
                    tmpB = tpool.tile([128, D_OUT], BF16, tag="tmp")
                    nc.scalar.activation(tmpA[:, :], py[:, 0:D_OUT],
                                         AF.Copy, scale=swA)
                    nc.scalar.activation(tmpB[:, :], py[:, D_OUT:2 * D_OUT],
                                         AF.Copy, scale=swB)
                    last = (ncx == N_NC - 1)
                    pending_adds.setdefault(ncx, []).append(
                        (dst, tmpA, False, False))
                    pending_adds.setdefault(ncx, []).append(
                        (dst, tmpB, False, last))

            def flush_adds(ncx):
                for dst, tmp, start, stop in pending_adds.pop(ncx, []):
                    nc.tensor.matmul(dst[:, :], identb[:, :], tmp[:, :],
                                     start=start, stop=stop)

            # ======== A1 for both halves ================================
            for ns in range(NB2):
                stepA1(ns)

            # (no early leaf chunks: PE has slack; rings stay in main loop)
            early_pys = []

            # ======== routing chain =====================================
            # Ln and Exp live in different activation-table sets; batching
            # all Lns then all Exps per stage caps table reloads at 4.
            # sq = hTb^2 (DVE, bf16 2x mode)
            for ns in range(NB2):
                nsl = slice(ns * 512, (ns + 1) * 512)
                for mc2 in range(2):
                    csl = slice(mc2 * 2 * B_CORE + ns * 1024,
                                mc2 * 2 * B_CORE + (ns + 1) * 1024)
                    nc.vector.tensor_tensor(sq[:, csl], hTb[:, csl],
                                            hTb[:, csl], op=ALU.mult)
                nc.scalar.activation(sq_last[0:1, nsl], h512b[0:1, nsl],
                                     AF.Square, bias=zero_col[0:1, 0:1])

            # ss = |h|^2 per column; rhh = exp(-0.5 ln ss) = 1/|h|
            phhs = []
            for ns in range(NB2):
                nsl = slice(ns * 512, (ns + 1) * 512)
                phh = papool.tile([1, 512], F32, tag="pa", name="phh")
                for mc in range(5):
                    pc = 128 if mc < 4 else 1
                    rhs = (sq[:, mc * B_CORE + ns * 512:
                              mc * B_CORE + (ns + 1) * 512]
                           if mc < 4 else sq_last[0:1, nsl])
                    nc.tensor.matmul(phh[0:1, :], ones_col_b[0:pc, 0:1], rhs,
                                     start=(mc == 0), stop=(mc == 4))
                phhs.append(phh)
            for ns in range(NB2):
                nsl = slice(ns * 512, (ns + 1) * 512)
                nc.scalar.activation(rhh_ln[0:1, nsl], phhs[ns][0:1, :],
                                     AF.Ln, bias=zero_col[0:1, 0:1])
            nc.scalar.activation(rhh_row_b[0:1, :], rhh_ln[0:1, :],
                                 AF.Exp, scale=-0.5,
                                 bias=zero_col[0:1, 0:1])

            # cosine: cT = (nw . h) / |h|, clamped to +-0.98
            for ns in range(NB2):
                nsl = slice(ns * 512, (ns + 1) * 512)
                pbc = papool.tile([128, 512], F32, tag="pa", name="pbc")
                nc.tensor.matmul(pbc[:, :], ones_row_b[0:1, 0:128],
                                 rhh_row_b[0:1, nsl], start=True, stop=True)
                nc.scalar.activation(rhh_bc[:, nsl], pbc[:, :], AF.Copy)
                prT = papool.tile([128, 512], F32, tag="pa", name="prT")
                for kt in range(5):
                    pc = 128 if kt < 4 else 1
                    lhsT = (nwt_sb[:, kt * NUM_NODES:(kt + 1) * NUM_NODES]
                            if kt < 4 else nwt_last[0:1, :])
                    rhs = (hTb[:, kt * B_CORE + ns * 512:
                               kt * B_CORE + (ns + 1) * 512]
                           if kt < 4 else h512b[0:1, nsl])
                    nc.tensor.matmul(prT[0:NUM_NODES, :], lhsT, rhs,
                                     start=(kt == 0), stop=(kt == 4))
                nc.vector.tensor_tensor(cT[0:NUM_NODES, nsl],
                                        prT[0:NUM_NODES, :],
                                        rhh_bc[0:NUM_NODES, nsl], op=ALU.mult)
                # clamp cosine to [-0.98, 0.98] (== prob clip [0.01, 0.99])
                nc.vector.tensor_scalar(cT[0:NUM_NODES, nsl],
                                        cT[0:NUM_NODES, nsl],
                                        0.98, -0.98, op0=ALU.min, op1=ALU.max)

            # dh = hTb - hT8 (bf16; exact gap of the on-device quantization)
            for kt in range(N_KT):
                nc.vector.scalar_tensor_tensor(
                    dh[:, kt * B_CORE:(kt + 1) * B_CORE],
                    hT8[:, kt, :], -1.0,
                    hTb[:, kt * B_CORE:(kt + 1) * B_CORE],
                    op0=ALU.mult, op1=ALU.add)

            # tA' = ln(1-c), tB' = ln(1+c): log route probs + ln2, centered
            # (~+-0.03) so bf16 is exact enough; the -7*ln2 shift is
            # restored in the Exp bias / entropy stt below
            nc.scalar.activation(tA[0:NUM_NODES, :], cT[0:NUM_NODES, :],
                                 AF.Ln, scale=-1.0,
                                 bias=ones_col[0:NUM_NODES, 0:1])
            nc.scalar.activation(tB[0:NUM_NODES, :], cT[0:NUM_NODES, :],
                                 AF.Ln, scale=1.0,
                                 bias=ones_col[0:NUM_NODES, 0:1])

            # ======== accumulator init: bias + col512 + fp8 correction ===
            def init_matmuls(dst, bt, close):
                bsl = slice(bt * 128, (bt + 1) * 128)
                nc.tensor.matmul(dst[:, :], swTb[:, bsl], wb_sb[:, :],
                                 start=True, stop=False)
                nc.tensor.matmul(dst[:, :], sw512Tb[:, bsl], wc5_sb[:, :],
                                 start=False, stop=False)
                for kt in range(N_KT):
                    ksl = slice(kt * B_CORE + bt * 128,
                                kt * B_CORE + (bt + 1) * 128)
                    osl = slice(kt * D_OUT, (kt + 1) * D_OUT)
                    nc.tensor.matmul(dst[:, :], hTb[:, ksl], ca_sb[:, osl],
                                     start=False, stop=False)
                    nc.tensor.matmul(dst[:, :], dh[:, ksl], cb_sb[:, osl],
                                     start=False,
                                     stop=(close and kt == N_KT - 1))

            # per-half: softmax-weights, transposes, and accumulator inits
            # all issue as soon as that half's routing chain finishes
            for ns in range(NB2):
                nsl = slice(ns * 512, (ns + 1) * 512)
                plp = papool.tile([128, 512], F32, tag="pa", name="plp")
                nc.tensor.matmul(plp[:, :], m_sb[:, 0:128], tA[:, nsl],
                                 start=True, stop=False)
                nc.tensor.matmul(plp[:, :], m_sb[:, 128:256], tB[:, nsl],
                                 start=False, stop=True)
                nc.scalar.activation(swT[:, nsl], plp[:, :], AF.Exp,
                                     bias=nl128_col[:, 0:1])
                nc.vector.tensor_copy(swTb[:, nsl], swT[:, nsl])
                nc.vector.scalar_tensor_tensor(entt[:, nsl], plp[:, :],
                                               -LN128, swT[:, nsl],
                                               op0=ALU.add, op1=ALU.mult)
                pent = papool.tile([1, 512], F32, tag="pa", name="pent")
                nc.tensor.matmul(pent[0:1, :], ones_col_b[:, 0:1],
                                 entt[:, nsl], start=True, stop=True)
                nc.vector.reduce_sum(srow[0:1, ns:ns + 1], pent[0:1, :],
                                     axis=mybir.AxisListType.X)
                # h512 broadcast to 128 partitions; sw512T = swT * h512
                pb5 = papool.tile([128, 512], F32, tag="pa", name="pb5")
                nc.tensor.matmul(pb5[:, :], ones_row_b[0:1, 0:128],
                                 h512b[0:1, nsl], start=True, stop=True)
                nc.scalar.activation(h512bc[:, nsl], pb5[:, :], AF.Copy)
                nc.vector.tensor_tensor(sw512Tb[:, nsl], swT[:, nsl],
                                        h512bc[:, nsl], op=ALU.mult)
                # sw_all = transpose(swT) * 2^-6 (undo the x64 W scale)
                for bt in range(4 * ns, 4 * ns + 4):
                    sl = slice(bt * 128, (bt + 1) * 128)
                    pt = papool.tile([128, 128], F32, tag="pa", name="pt")
                    nc.tensor.matmul(pt[:, :], swT[:, sl], ident[:, :],
                                     start=True, stop=True)
                    nc.scalar.activation(sw_all[:, sl], pt[:, :], AF.Copy,
                                         scale=1.0 / W_SCALE)
                for bt in range(4 * ns, 4 * ns + 4):
                    if bt in ACT_BTS:
                        # feeds the persistent PSUM accumulator group
                        # directly; closed by the last identity-add
                        init_matmuls(accP[bt], bt, close=False)
                    elif bt == 5:
                        accP5 = initpool.tile([128, D_OUT], F32, tag="init",
                                              name="accP5")
                        init_matmuls(accP5, bt, close=False)
                    else:
                        pinit = initpool.tile([128, D_OUT], F32, tag="init",
                                              name="pinit")
                        init_matmuls(pinit, bt, close=True)
                        nc.vector.tensor_copy(
                            acc_d[:, bt * D_OUT:(bt + 1) * D_OUT],
                            pinit[:, :])

            # ======== allreduce of entropy partials -> output scale ======
            ccin = dram.tile([1, 8], F32)
            ccout = dram.tile([1, 8], F32)
            nc.sync.dma_start(ccin[:], srow[0:1, :])
            if with_collective:
                nc.gpsimd.collective_compute(
                    "AllReduce", ALU.add,
                    replica_groups=[list(range(N_CORES))],
                    ins=[ccin.opt()], outs=[ccout.opt()])
                nc.sync.dma_start(stot[0:1, :], ccout[:])
            else:
                # single-core sim variant: no collective
                nc.sync.dma_start(stot[0:1, :], ccin[:])


            # ======== main loop: leaf matmuls + combines ================
            for ncx in range(N_NC):
                pys = early_pys[ncx] if ncx < N_EARLY else leaf_matmuls(ncx)
                for bt in BT_ORDER:
                    combine(ncx, bt, pys[bt], accP5)
                if ncx >= ADD_LAG:
                    flush_adds(ncx - ADD_LAG)
            for ncx in range(N_NC - ADD_LAG, N_NC):
                flush_adds(ncx)

            # scale = 1 - (S0+S1) / (BATCH * MAX_ENT) -> [128,1]; done
            # after the main loop so the papool ring never waits on the
            # collective result
            nc.vector.reduce_sum(scal[0:1, 0:1], stot[0:1, 0:2],
                                 axis=mybir.AxisListType.X)
            nc.vector.tensor_scalar(scal[0:1, 0:1], scal[0:1, 0:1],
                                    -1.0 / (BATCH * MAX_ENT), 1.0,
                                    op0=ALU.mult, op1=ALU.add)
            psc = papool.tile([128, 1], F32, tag="pa", name="psc")
            nc.tensor.matmul(psc[:, 0:1], ones_row[0:1, :], scal[0:1, 0:1],
                             start=True, stop=True)
            nc.scalar.activation(scol[:, 0:1], psc[:, 0:1], AF.Copy)

            # ======== tail: merge accumulators, scale, store ============
            for bt in range(N_BT):
                ot = opool.tile([128, D_OUT], F32, tag="o")
                if bt >= 5:
                    src_acc = accP5 if bt == 5 else accP[bt]
                    nc.scalar.activation(ot[:, :], src_acc[:, :],
                                         AF.Copy, scale=scol[:, 0:1])
                else:
                    # acc_d holds init (+ DVE combines), acc_p the Pool
                    # combines (zero for bt 2-4)
                    asl_d = acc_d[:, bt * D_OUT:(bt + 1) * D_OUT]
                    asl_p = acc_p[:, bt * D_OUT:(bt + 1) * D_OUT]
                    nc.vector.tensor_tensor(asl_d, asl_d, asl_p, op=ALU.add)
                    nc.scalar.activation(ot[:, :], asl_d,
                                         AF.Copy, scale=scol[:, 0:1])
                nc.sync.dma_start(out_c[bt * 128:(bt + 1) * 128, :],
                                  ot[:, :])

    nc.compile()
    return nc


def _prep_inputs(x, W_pre, b_pre, right_w, W_leaf, b_leaf, route_idx,
                 route_side):
    x = np.asarray(x, np.float32)
    W_pre = np.asarray(W_pre, np.float32)
    b_pre = np.asarray(b_pre, np.float32)
    right_w = np.asarray(right_w, np.float32)
    W_leaf = np.asarray(W_leaf, np.float32)
    b_leaf = np.asarray(b_leaf, np.float32)
    route_idx = np.asarray(route_idx)
    route_side = np.asarray(route_side)

    xT = np.ascontiguousarray(x.T).astype(ml_dtypes.bfloat16)  # [512, 8192]
    wpre = np.ascontiguousarray(W_pre.T).astype(ml_dtypes.bfloat16)
    bpre = np.ascontiguousarray(b_pre.reshape(D1, 1))
    nw = right_w / np.maximum(
        np.linalg.norm(right_w, axis=1, keepdims=True), 1e-12)
    nwt = np.ascontiguousarray(nw.T).astype(ml_dtypes.bfloat16)  # [513, 127]

    M = np.zeros((256, 128), np.float32)
    n_steps = route_idx.shape[1]
    for leaf in range(NUM_LEAVES):
        for d in range(n_steps):
            node = int(route_idx[leaf, d])
            side = int(route_side[leaf, d])
            M[node + (128 if side else 0), leaf] += 1.0

    wlT = np.ascontiguousarray(W_leaf[:, :D_IN].T)      # [512, 32768] f32
    wleaf8 = (wlT * W_SCALE).astype(ml_dtypes.float8_e4m3)
    # exact-mean correction matrices (fp8 quantization noise cancellation)
    wbar = wlT.reshape(D_IN, NUM_LEAVES, D_OUT).mean(axis=1)        # [512,256]
    w8bar = (wleaf8.astype(np.float32) / W_SCALE).reshape(
        D_IN, NUM_LEAVES, D_OUT).mean(axis=1)
    ca = (wbar - w8bar).astype(ml_dtypes.bfloat16)      # h @ (Wbar - W8bar)
    cb = w8bar.astype(ml_dtypes.bfloat16)               # dh @ W8bar

    wbias = np.ascontiguousarray(
        b_leaf.reshape(NUM_LEAVES, D_OUT)).astype(ml_dtypes.bfloat16)
    wcol512 = np.ascontiguousarray(
        W_leaf[:, D_IN].reshape(NUM_LEAVES, D_OUT)).astype(ml_dtypes.bfloat16)

    shared = {"wpre": wpre, "bpre": bpre, "nwt": nwt,
              "mroute": M.astype(ml_dtypes.bfloat16),
              "wleaf8": wleaf8, "wbias": wbias, "wcol512": wcol512,
              "corr_a": ca, "corr_b": cb}
    in_maps = []
    for c in range(N_CORES):
        m = dict(shared)
        m["xT_c"] = np.ascontiguousarray(
            xT[:, c * B_CORE:(c + 1) * B_CORE])
        in_maps.append(m)
    return in_maps


def kernel(x, W_pre, b_pre, right_w, W_leaf, b_leaf, route_idx, route_side):
    global _COMPILED
    if _COMPILED is None:
        _COMPILED = _build_program()
    nc = _COMPILED
    in_maps = _prep_inputs(x, W_pre, b_pre, right_w, W_leaf, b_leaf,
                           route_idx, route_side)
    res = run_bass_kernel_spmd(nc, in_maps, core_ids=list(range(N_CORES)))
    out = np.concatenate([res.results[c]["out_c"] for c in range(N_CORES)],
                         axis=0)
    return out.astype(np.float32)
